# revision 36
# baseline (speedup 1.0000x reference)
"""Causal attention decoder block on 8 trn2 NeuronCores.

Sharding: core = (batch b in 0..1, head-group g in 0..3); each core computes
4 heads of one batch element: QKV projection slices, RoPE, causal attention,
and a partial output projection (its heads' rows of Wout). Host sums the 4
partials per batch and adds bout.

Device schedule notes (v2 - PE-saturation rewrite):
  - The tensor engine's clock ramps with sustained use (1.2GHz -> 2.4GHz after
    ~3us continuous busy). The whole kernel is emitted as one interleaved PE
    stream: attention (scores/PV) is software-pipelined (PV lags scores by 2
    m-tiles so the scalar-engine exp is off the critical path) and independent
    projection matmuls (remaining QK chunks, V tiles, output projection of the
    previous q-chunk) are injected as fillers between PV steps.
  - Triangular causal mask is added in PSUM via a bf16 identity@tri matmul
    (1 cycle/row vs 4 for fp32).
  - Scalar engine does (almost) only exp; PSUM evacuations go to gpsimd,
    normalization reciprocal uses the fast DVE approx op.
  - q-chunk order [1,2,3,0] so the last attention phase is the smallest and
    the tail out-projection is cheap.
"""
import ml_dtypes
import numpy as np

import concourse.bass as bass
import concourse.mybir as mybir
from concourse import bacc
from concourse.ap import AP
from concourse.tile import TileContext

F32 = mybir.dt.float32
F32R = mybir.dt.float32r
BF16 = mybir.dt.bfloat16
EXP = mybir.ActivationFunctionType.Exp

B, N, D = 2, 2048, 1024
H, HD = 16, 64
HPG = 4               # heads per group/core
C = HPG * HD          # 256 cols per core per tensor
SCALE = HD ** -0.5
ROPE_BASE = 10000.0
NT = N // 128         # 16 seq tiles
NCH = N // 512        # 4 seq chunks
KT = D // 128         # 8 contraction tiles
MBIG = -1e9
LAG = 3               # PV lags scores by this many m-tiles

# ---------------------------------------------------------------- host tables

def _host_tables():
    perm = np.zeros(HD, np.int64)
    freqi = np.zeros(HD, np.int64)
    sign = np.zeros(HD, np.float32)
    for c in range(HD):
        q, r = divmod(c, 32)
        s, j = divmod(r, 16)
        i = q * 16 + j
        perm[c] = 2 * i + s
        freqi[c] = i
        sign[c] = -1.0 if s == 0 else 1.0
    inv_freq = 1.0 / (ROPE_BASE ** (np.arange(0, HD, 2, dtype=np.float32) / HD))
    ang = np.outer(inv_freq[freqi], np.arange(N, dtype=np.float32))   # (64, N)
    cos2 = np.tile(np.cos(ang).astype(np.float32), (2, 1))            # (128, N)
    sin2 = np.tile((np.sin(ang) * sign[:, None]).astype(np.float32), (2, 1))
    # triangular tile: element (m, q) masks scores with q < m
    m = np.arange(128)[:, None]
    q = np.arange(128)[None, :]
    tri = np.where(q >= m, 0.0, MBIG).astype(np.float32)
    ident = np.eye(128, dtype=np.float32)
    return perm, cos2, sin2, tri, ident

_PERM, _COS2, _SIN2, _TRI, _IDENT = _host_tables()
_SHUF_MASK = [(i ^ 16) for i in range(32)]
# selector for broadcasting the per-chunk sums collector (4 rows, row = head)
# to a 128-partition head-pair tile: block t rows 0-63 <- head 2t, 64-127 <-
# head 2t+1
_SEL = np.zeros((4, 256), np.float32)
for _t in range(2):
    _SEL[2 * _t, _t * 128:_t * 128 + 64] = 1.0
    _SEL[2 * _t + 1, _t * 128 + 64:_t * 128 + 128] = 1.0

# ---------------------------------------------------------------- bass kernel

def build_nc():
    nc = bacc.Bacc("TRN2", target_bir_lowering=False, debug=False)
    xt_d = nc.dram_tensor("xt", [D, N], BF16, kind="ExternalInput").ap()
    wq_d = nc.dram_tensor("wq", [D, C], BF16, kind="ExternalInput").ap()
    wk_d = nc.dram_tensor("wk", [D, C], BF16, kind="ExternalInput").ap()
    wv_d = nc.dram_tensor("wv", [D, C], BF16, kind="ExternalInput").ap()
    wout_d = nc.dram_tensor("wout", [C, D], BF16, kind="ExternalInput").ap()
    cos_d = nc.dram_tensor("cos2", [128, N], BF16, kind="ExternalInput").ap()
    sin_d = nc.dram_tensor("sin2", [128, N], BF16, kind="ExternalInput").ap()
    tri_d = nc.dram_tensor("tri", [128, 128], BF16, kind="ExternalInput").ap()
    id_d = nc.dram_tensor("ident", [128, 128], BF16, kind="ExternalInput").ap()
    sel_d = nc.dram_tensor("sel", [4, 256], F32R, kind="ExternalInput").ap()
    out_d = nc.dram_tensor("out", [N, D], F32, kind="ExternalOutput").ap()

    with TileContext(nc) as tc:
        with tc.tile_pool(name="persist", bufs=1) as pp, \
             tc.tile_pool(name="xt", bufs=1) as xp, \
             tc.tile_pool(name="scr", bufs=4) as sp, \
             tc.tile_pool(name="ps_s", bufs=4, space="PSUM") as spool, \
             tc.tile_pool(name="ps_f", bufs=2, space="PSUM") as fpool, \
             tc.tile_pool(name="ps_pv", bufs=2, space="PSUM") as pvp:

            # ---------------- DMA loads: one batched DMA per tensor/chunk so
            # descriptor-gen doesn't serialize the startup (k-tile k of a
            # [D, W] DRAM tensor lands at columns [k*W:(k+1)*W] of one wide
            # [128, KT*W] SBUF tile)
            def load_blocked(host, dst_tile, src_ap, w, nblk, c0=0, cw=None,
                             k0=0, pitch=None):
                cw = w if cw is None else cw
                pitch = nblk * w if pitch is None else pitch
                da = dst_tile[:]
                dst = AP(da.tensor, da.offset + c0 + k0 * w,
                         [[pitch, 128], [w, nblk], [1, cw]])
                src = AP(src_ap.tensor, src_ap.offset + c0 + k0 * 128 * w,
                         [[w, 128], [128 * w, nblk], [1, cw]])
                host.dma_start(dst, src)

            wq_all = pp.tile([128, KT * C], BF16, tag="wq", name="wq")
            wk_all = pp.tile([128, KT * C], BF16, tag="wk", name="wk")
            wv_all = pp.tile([128, KT * C], BF16, tag="wv", name="wv")
            xt_all = xp.tile([128, KT * N], BF16, tag="xt", name="xt")
            wout_all = pp.tile([128, 2 * D], BF16, tag="wout", name="wout")
            cos_sb = pp.tile([128, N], BF16, tag="cos")
            sin_sb = pp.tile([128, N], BF16, tag="sin")
            tri_sb = pp.tile([128, 128], BF16, tag="tri")
            id_sb = pp.tile([128, 128], BF16, tag="ident")
            sel_sb = pp.tile([4, 256], F32R, tag="sel")

            # priority: phase A touches only wq/wk/xt-ch0 (+rope tables).
            # sync/scalar host HARDWARE dge queues (first bytes ~8us);
            # gpsimd's queue is software-dge (~13us startup) so it only
            # gets loads needed later.
            load_blocked(nc.sync, wq_all, wq_d, C, KT)
            load_blocked(nc.scalar, wk_all, wk_d, C, KT)
            # ch0 split by k-halves across both HW-dge queues (gpsimd's
            # software-dge queue takes ~13us to produce its first bytes)
            load_blocked(nc.sync, xt_all, xt_d, N, 4, 0 * 512, 512, k0=0,
                         pitch=KT * N)
            load_blocked(nc.scalar, xt_all, xt_d, N, 4, 0 * 512, 512, k0=4,
                         pitch=KT * N)
            nc.sync.dma_start(cos_sb[:], cos_d[:])
            nc.scalar.dma_start(sin_sb[:], sin_d[:])
            load_blocked(nc.gpsimd, wv_all, wv_d, C, KT)
            load_blocked(nc.sync, xt_all, xt_d, N, KT, 1 * 512, 512)
            nc.scalar.dma_start(tri_sb[:], tri_d[:])
            nc.scalar.dma_start(id_sb[:], id_d[:])
            nc.scalar.dma_start(sel_sb[:], sel_d[:])
            load_blocked(nc.gpsimd, xt_all, xt_d, N, KT, 2 * 512, 512)
            load_blocked(nc.sync, xt_all, xt_d, N, KT, 3 * 512, 512)
            load_blocked(nc.gpsimd, wout_all, wout_d, D, 2)

            def xtk(k, a, b):
                return xt_all[:, k * N + a:k * N + b]

            # ---------------- persistent SBUF results
            qr_sb = [pp.tile([128, N], BF16, tag=f"qr{t}", name=f"qr{t}")
                     for t in range(2)]
            kr_sb = [pp.tile([128, N], BF16, tag=f"kr{t}", name=f"kr{t}")
                     for t in range(2)]
            vaug_sb = [pp.tile([128, HPG * (HD + 1)], BF16, tag=f"va{i}",
                               name=f"va{i}") for i in range(NT)]
            ou_sb = [pp.tile([128, N], BF16, tag=f"ou{t}", name=f"ou{t}")
                     for t in range(2)]
            o_sb = [pp.tile([128, N], BF16, tag=f"o{t}", name=f"o{t}")
                    for t in range(2)]
            sums_sb = [pp.tile([4, 512], F32, tag=f"sums{qc}", name=f"sums{qc}")
                       for qc in range(NCH)]
            # ones columns of vaug written once
            for i in range(NT):
                ap = vaug_sb[i][:]
                dst1 = AP(ap.tensor, ap.offset + HD,
                          [[HPG * (HD + 1), 128], [HD + 1, HPG]])
                nc.gpsimd.memset(dst1, 1.0)

            # ---------------- PE work generators
            def gen_qk(w_all, dst, mt, ch):
                ps = fpool.tile([128, 512], F32, tag="fill", name="qkps")
                for k in range(KT):
                    nc.tensor.matmul(
                        ps[:],
                        w_all[:, k * C + mt * 128:k * C + (mt + 1) * 128],
                        xtk(k, ch * 512, (ch + 1) * 512),
                        start=(k == 0), stop=(k == KT - 1))
                    yield
                cs = cos_sb[:, ch * 512:(ch + 1) * 512]
                sn = sin_sb[:, ch * 512:(ch + 1) * 512]
                xs = sp.tile([128, 512], F32, tag="xs", name="xs", bufs=2)
                nc.vector.stream_shuffle(xs[:], ps[:], _SHUF_MASK)
                m2 = sp.tile([128, 512], F32, tag="mm", name="m2")
                nc.vector.tensor_mul(m2[:], xs[:], sn)
                m1 = sp.tile([128, 512], F32, tag="mm", name="m1")
                nc.vector.tensor_mul(m1[:], ps[:], cs)
                nc.vector.tensor_add(
                    dst[mt][:, ch * 512:(ch + 1) * 512], m1[:], m2[:])

            def gen_v(i):
                ps = fpool.tile([128, C], F32, tag="fill", name="vps")
                for k in range(KT):
                    nc.tensor.matmul(
                        ps[:], xtk(k, i * 128, (i + 1) * 128),
                        wv_all[:, k * C:(k + 1) * C],
                        start=(k == 0), stop=(k == KT - 1))
                    yield
                ap = vaug_sb[i][:]
                dst = AP(ap.tensor, ap.offset,
                         [[HPG * (HD + 1), 128], [HD + 1, HPG], [1, HD]])
                # gpsimd cannot read PSUM: scalar is idle during phase A
                # (tiles 0-7), DVE takes the mid-attention ones (8-15)
                if i < 8:
                    nc.scalar.copy(dst, ps[:].rearrange("p (a c) -> p a c",
                                                        a=HPG, c=HD))
                else:
                    nc.vector.tensor_copy(dst, ps[:].rearrange(
                        "p (a c) -> p a c", a=HPG, c=HD))

            def gen_norm(qc):
                # recip + cast emitted immediately (DVE/scalar); the PE bc
                # matmuls + DVE muls come as two filler quanta.
                rrf = sp.tile([4, 512], F32, tag="rrf", name="rrf", bufs=2)
                nc.vector.reciprocal_approx_fast(rrf[:], sums_sb[qc][:])
                rr = sp.tile([4, 512], F32R, tag="rr", name="rr", bufs=2)
                nc.scalar.copy(rr[:], rrf[:])
                for t in range(2):
                    yield
                    bc = fpool.tile([128, 512], F32, tag="fill", name="bc")
                    nc.tensor.matmul(bc[:], sel_sb[:, t * 128:(t + 1) * 128],
                                     rr[:], start=True, stop=True)
                    nc.vector.tensor_mul(
                        o_sb[t][:, qc * 512:(qc + 1) * 512],
                        ou_sb[t][:, qc * 512:(qc + 1) * 512], bc[:])

            def gen_op(qc, ilist):
                for i in ilist:
                    for cc in range(2):
                        ps = fpool.tile([128, 512], F32, tag="fill", name="ops")
                        for t in range(2):
                            nc.tensor.matmul(
                                ps[:],
                                o_sb[t][:, i * 128:(i + 1) * 128],
                                wout_all[:, t * D + cc * 512:
                                         t * D + (cc + 1) * 512],
                                start=(t == 0), stop=(t == 1))
                        oc = sp.tile([128, 512], F32, tag="oc", name="oc",
                                     bufs=3)
                        # qc 3 runs in the scalar-idle tail: alternate
                        # scalar/vector there so the copies overlap
                        if qc == 3 and (i * 2 + cc) % 2 == 0:
                            nc.scalar.copy(oc[:], ps[:])
                        else:
                            nc.vector.tensor_copy(oc[:], ps[:])
                        # alternate output-DMA queues so the final drain
                        # isn't serialized on one queue
                        dma_h = nc.sync if (i * 2 + cc) % 2 == 0 else nc.gpsimd
                        dma_h.dma_start(
                            out_d[i * 128:(i + 1) * 128,
                                  cc * 512:(cc + 1) * 512], oc[:])
                        yield

            # filler machinery: list of generators, pulled n quanta at a time
            def mk_fill(gens):
                gens = list(gens)
                def fill(n):
                    while n > 0 and gens:
                        try:
                            next(gens[0])
                            n -= 1
                        except StopIteration:
                            gens.pop(0)
                def drain():
                    while gens:
                        try:
                            next(gens[0])
                        except StopIteration:
                            gens.pop(0)
                return fill, drain

            def run(gen):  # run a generator to completion (dense emission)
                for _ in gen:
                    pass

            # ---------------- attention head with PV lag + fillers
            def attention_head(qc, hl, fill, rate=2):
                t = hl // 2
                pb = (hl % 2) * 64
                nmt = 4 * (qc + 1)
                pv = pvp.tile([HD + 1, 512], F32, tag="pv", name="pv")
                pend = []

                def emit_pv(e_sb, q0, mt):
                    nc.tensor.matmul(
                        pv[:, q0:512],
                        vaug_sb[mt][:, hl * (HD + 1):(hl + 1) * (HD + 1)],
                        e_sb[:, q0:512],
                        start=(mt == 0), stop=(mt == nmt - 1))

                for mt in range(nmt):
                    v = mt - 4 * qc          # >=0 on diagonal m-tiles
                    q0 = 128 * v if v > 0 else 0
                    s_ps = spool.tile([128, 512], F32, tag="s", name="sps")
                    nc.tensor.matmul(
                        s_ps[:, q0:512],
                        kr_sb[t][pb:pb + 64, mt * 128:(mt + 1) * 128],
                        qr_sb[t][pb:pb + 64, qc * 512 + q0:(qc + 1) * 512],
                        start=True, stop=(v < 0))
                    if v >= 0:
                        nc.tensor.matmul(
                            s_ps[:, q0:q0 + 128], id_sb[:], tri_sb[:],
                            start=False, stop=True)
                    e_sb = sp.tile([128, 512], BF16, tag="e", name="e", bufs=6)
                    nc.scalar.activation(e_sb[:, q0:512], s_ps[:, q0:512],
                                         EXP, scale=SCALE)
                    pend.append((e_sb, q0, mt))
                    if len(pend) > LAG:
                        emit_pv(*pend.pop(0))
                        fill(rate)
                while pend:
                    emit_pv(*pend.pop(0))
                    fill(rate)
                # evacuate sums row + unnormalized output (DVE; gpsimd has no
                # PSUM access, engines can only write partition bases
                # 0/32/64/96 so the row bounces through partition 0 + DMA)
                sr = sp.tile([1, 512], F32, tag="sr", name="sr", bufs=2)
                nc.vector.tensor_copy(sr[:], pv[64:65, :])
                nc.sync.dma_start(sums_sb[qc][hl:hl + 1, :], sr[:])
                nc.vector.tensor_copy(
                    ou_sb[t][pb:pb + 64, qc * 512:(qc + 1) * 512],
                    pv[0:64, :])

            def gen_delay(n):
                for _ in range(n):
                    yield

            # ---------------- phase A: dense warm-up touching only xt ch0
            # first, then ch1, so At[1, heads 0/1] can start ASAP
            run(gen_qk(wq_all, qr_sb, 0, 0))
            run(gen_qk(wk_all, kr_sb, 0, 0))
            for i in range(4):
                run(gen_v(i))
            run(gen_qk(wq_all, qr_sb, 0, 1))
            run(gen_qk(wk_all, kr_sb, 0, 1))
            for i in range(4, 8):
                run(gen_v(i))

            # ---------------- interleaved main phases; q-chunk order 1,2,3
            # with the small qc=0 heads woven into the under-filled qc=3
            # phase, so the only tail is norm3+op3
            plan = {
                (1, 0): [gen_qk(wq_all, qr_sb, 1, 0), gen_qk(wk_all, kr_sb, 1, 0)],
                (1, 1): [gen_qk(wq_all, qr_sb, 1, 1), gen_qk(wk_all, kr_sb, 1, 1)],
                (1, 2): [gen_qk(wq_all, qr_sb, 0, 2), gen_qk(wk_all, kr_sb, 0, 2)],
                (1, 3): [gen_qk(wq_all, qr_sb, 1, 2), gen_qk(wk_all, kr_sb, 1, 2)],
                (2, 0): [gen_v(8), gen_v(9), gen_v(10), gen_v(11), "norm1"],
                (2, 1): [gen_qk(wq_all, qr_sb, 0, 3), gen_qk(wk_all, kr_sb, 0, 3)],
                (2, 2): [gen_qk(wq_all, qr_sb, 1, 3), gen_qk(wk_all, kr_sb, 1, 3),
                         "op1a"],
                (2, 3): ["op1b"],
                (3, 0): [gen_v(12), gen_v(13), gen_v(14), gen_v(15), "norm2"],
                (0, 0): ["op2a"],
                (3, 1): ["op2b"],
                (0, 1): [],
                (0, 2): [],
                (0, 3): [],
                (3, 2): [gen_delay(6), "norm0"],
                (3, 3): ["op0a", "op0b"],
            }
            named = {
                "norm0": lambda: gen_norm(0),
                "norm1": lambda: gen_norm(1),
                "norm2": lambda: gen_norm(2),
                "op0a": lambda: gen_op(0, [0, 1]),
                "op0b": lambda: gen_op(0, [2, 3]),
                "op1a": lambda: gen_op(1, [4, 5]),
                "op1b": lambda: gen_op(1, [6, 7]),
                "op2a": lambda: gen_op(2, [8, 9]),
                "op2b": lambda: gen_op(2, [10, 11]),
            }
            # higher pull-rate where same-head V-proj fillers must finish
            # before the pv matmuls that consume them (see deadlock audit)
            rates = {(2, 0): 4, (3, 0): 3, (3, 2): 1}
            order = [(1, 0), (1, 1), (1, 2), (1, 3),
                     (2, 0), (2, 1), (2, 2), (2, 3),
                     (3, 0), (0, 0), (3, 1), (0, 1),
                     (0, 2), (0, 3), (3, 2), (3, 3)]
            for qc, hl in order:
                gens = [named[g]() if isinstance(g, str) else g
                        for g in plan[(qc, hl)]]
                fill, drain = mk_fill(gens)
                attention_head(qc, hl, fill, rate=rates.get((qc, hl), 2))
                drain()
            # tail: normalize + project chunk 3
            run(gen_norm(3))
            run(gen_op(3, [12, 13, 14, 15]))

    nc.compile()
    return nc


# ---------------------------------------------------------------- host wrapper

_NC = None


def make_in_maps(X, Wqkv, Wout, bout):
    X = np.ascontiguousarray(np.asarray(X, np.float32))
    Wqkv = np.asarray(Wqkv, np.float32)
    Wout = np.asarray(Wout, np.float32)
    in_maps = []
    for core in range(8):
        b, g = core // 4, core % 4
        heads = [HPG * g + hl for hl in range(HPG)]
        qcols = np.concatenate([h * HD + _PERM for h in heads])
        vcols = np.concatenate([h * HD + np.arange(HD) for h in heads])
        in_maps.append({
            "xt": np.ascontiguousarray(X[b].T).astype(ml_dtypes.bfloat16),
            "wq": np.ascontiguousarray(Wqkv[:, qcols]).astype(ml_dtypes.bfloat16),
            "wk": np.ascontiguousarray(Wqkv[:, 1024 + qcols]).astype(ml_dtypes.bfloat16),
            "wv": np.ascontiguousarray(Wqkv[:, 2048 + vcols]).astype(ml_dtypes.bfloat16),
            "wout": np.ascontiguousarray(Wout[vcols, :]).astype(ml_dtypes.bfloat16),
            "cos2": _COS2.astype(ml_dtypes.bfloat16),
            "sin2": _SIN2.astype(ml_dtypes.bfloat16),
            "tri": _TRI.astype(ml_dtypes.bfloat16),
            "ident": _IDENT.astype(ml_dtypes.bfloat16),
            "sel": _SEL,
        })
    return in_maps


def assemble(results, bout):
    out = np.zeros((B, N, D), np.float32)
    for core in range(8):
        out[core // 4] += results[core]["out"]
    out += np.asarray(bout, np.float32)[None, None, :]
    return out


def kernel(X, Wqkv, Wout, bout):
    global _NC
    from concourse import bass_utils
    if _NC is None:
        _NC = build_nc()
    in_maps = make_in_maps(X, Wqkv, Wout, bout)
    res = bass_utils.run_bass_kernel_spmd(_NC, in_maps, core_ids=list(range(8)))
    return assemble(res.results, bout)


# revision 37
# speedup vs baseline: 1.0232x; 1.0232x over previous
"""Causal attention decoder block on 8 trn2 NeuronCores.

Sharding: core = (batch b in 0..1, head-group g in 0..3); each core computes
4 heads of one batch element: QKV projection slices, RoPE, causal attention,
and a partial output projection (its heads' rows of Wout). Host sums the 4
partials per batch and adds bout.

Device schedule notes (v2 - PE-saturation rewrite):
  - The tensor engine's clock ramps with sustained use (1.2GHz -> 2.4GHz after
    ~3us continuous busy). The whole kernel is emitted as one interleaved PE
    stream: attention (scores/PV) is software-pipelined (PV lags scores by 2
    m-tiles so the scalar-engine exp is off the critical path) and independent
    projection matmuls (remaining QK chunks, V tiles, output projection of the
    previous q-chunk) are injected as fillers between PV steps.
  - Triangular causal mask is added in PSUM via a bf16 identity@tri matmul
    (1 cycle/row vs 4 for fp32).
  - Scalar engine does (almost) only exp; PSUM evacuations go to gpsimd,
    normalization reciprocal uses the fast DVE approx op.
  - q-chunk order [1,2,3,0] so the last attention phase is the smallest and
    the tail out-projection is cheap.
"""
import ml_dtypes
import numpy as np

import concourse.bass as bass
import concourse.mybir as mybir
from concourse import bacc
from concourse.ap import AP
from concourse.tile import TileContext

F32 = mybir.dt.float32
F32R = mybir.dt.float32r
BF16 = mybir.dt.bfloat16
EXP = mybir.ActivationFunctionType.Exp

B, N, D = 2, 2048, 1024
H, HD = 16, 64
HPG = 4               # heads per group/core
C = HPG * HD          # 256 cols per core per tensor
SCALE = HD ** -0.5
ROPE_BASE = 10000.0
NT = N // 128         # 16 seq tiles
NCH = N // 512        # 4 seq chunks
KT = D // 128         # 8 contraction tiles
MBIG = -1e9
LAG = 3               # PV lags scores by this many m-tiles

# ---------------------------------------------------------------- host tables

def _host_tables():
    perm = np.zeros(HD, np.int64)
    freqi = np.zeros(HD, np.int64)
    sign = np.zeros(HD, np.float32)
    for c in range(HD):
        q, r = divmod(c, 32)
        s, j = divmod(r, 16)
        i = q * 16 + j
        perm[c] = 2 * i + s
        freqi[c] = i
        sign[c] = -1.0 if s == 0 else 1.0
    inv_freq = 1.0 / (ROPE_BASE ** (np.arange(0, HD, 2, dtype=np.float32) / HD))
    ang = np.outer(inv_freq[freqi], np.arange(N, dtype=np.float32))   # (64, N)
    cos2 = np.tile(np.cos(ang).astype(np.float32), (2, 1))            # (128, N)
    sin2 = np.tile((np.sin(ang) * sign[:, None]).astype(np.float32), (2, 1))
    # triangular tile: element (m, q) masks scores with q < m
    m = np.arange(128)[:, None]
    q = np.arange(128)[None, :]
    tri = np.where(q >= m, 0.0, MBIG).astype(np.float32)
    ident = np.eye(128, dtype=np.float32)
    return perm, cos2, sin2, tri, ident

_PERM, _COS2, _SIN2, _TRI, _IDENT = _host_tables()
_SHUF_MASK = [(i ^ 16) for i in range(32)]
# selector for broadcasting the per-chunk sums collector (4 rows, row = head)
# to a 128-partition head-pair tile: block t rows 0-63 <- head 2t, 64-127 <-
# head 2t+1
_SEL = np.zeros((4, 256), np.float32)
for _t in range(2):
    _SEL[2 * _t, _t * 128:_t * 128 + 64] = 1.0
    _SEL[2 * _t + 1, _t * 128 + 64:_t * 128 + 128] = 1.0

# ---------------------------------------------------------------- bass kernel

def build_nc():
    nc = bacc.Bacc("TRN2", target_bir_lowering=False, debug=False)
    xt_d = nc.dram_tensor("xt", [D, N], BF16, kind="ExternalInput").ap()
    wq_d = nc.dram_tensor("wq", [D, C], BF16, kind="ExternalInput").ap()
    wk_d = nc.dram_tensor("wk", [D, C], BF16, kind="ExternalInput").ap()
    wv_d = nc.dram_tensor("wv", [D, C], BF16, kind="ExternalInput").ap()
    wout_d = nc.dram_tensor("wout", [C, D], BF16, kind="ExternalInput").ap()
    cos_d = nc.dram_tensor("cos2", [128, N], BF16, kind="ExternalInput").ap()
    sin_d = nc.dram_tensor("sin2", [128, N], BF16, kind="ExternalInput").ap()
    tri_d = nc.dram_tensor("tri", [128, 128], BF16, kind="ExternalInput").ap()
    id_d = nc.dram_tensor("ident", [128, 128], BF16, kind="ExternalInput").ap()
    sel_d = nc.dram_tensor("sel", [4, 256], F32R, kind="ExternalInput").ap()
    out_d = nc.dram_tensor("out", [N, D], F32, kind="ExternalOutput").ap()

    with TileContext(nc) as tc:
        with tc.tile_pool(name="persist", bufs=1) as pp, \
             tc.tile_pool(name="xt", bufs=1) as xp, \
             tc.tile_pool(name="scr", bufs=4) as sp, \
             tc.tile_pool(name="ps_s", bufs=4, space="PSUM") as spool, \
             tc.tile_pool(name="ps_f", bufs=2, space="PSUM") as fpool, \
             tc.tile_pool(name="ps_pv", bufs=2, space="PSUM") as pvp:

            # ---------------- DMA loads: one batched DMA per tensor/chunk so
            # descriptor-gen doesn't serialize the startup (k-tile k of a
            # [D, W] DRAM tensor lands at columns [k*W:(k+1)*W] of one wide
            # [128, KT*W] SBUF tile)
            def load_blocked(host, dst_tile, src_ap, w, nblk, c0=0, cw=None,
                             k0=0, pitch=None):
                cw = w if cw is None else cw
                pitch = nblk * w if pitch is None else pitch
                da = dst_tile[:]
                dst = AP(da.tensor, da.offset + c0 + k0 * w,
                         [[pitch, 128], [w, nblk], [1, cw]])
                src = AP(src_ap.tensor, src_ap.offset + c0 + k0 * 128 * w,
                         [[w, 128], [128 * w, nblk], [1, cw]])
                host.dma_start(dst, src)

            wq_all = pp.tile([128, KT * C], BF16, tag="wq", name="wq")
            wk_all = pp.tile([128, KT * C], BF16, tag="wk", name="wk")
            wv_all = pp.tile([128, KT * C], BF16, tag="wv", name="wv")
            xt_all = xp.tile([128, KT * N], BF16, tag="xt", name="xt")
            wout_all = pp.tile([128, 2 * D], BF16, tag="wout", name="wout")
            cos_sb = pp.tile([128, N], BF16, tag="cos")
            sin_sb = pp.tile([128, N], BF16, tag="sin")
            tri_sb = pp.tile([128, 128], BF16, tag="tri")
            id_sb = pp.tile([128, 128], BF16, tag="ident")
            sel_sb = pp.tile([4, 256], F32R, tag="sel")

            # priority: phase A touches only wq/wk/xt-ch0 (+rope tables).
            # sync/scalar host HARDWARE dge queues (first bytes ~8us);
            # gpsimd's queue is software-dge (~13us startup) so it only
            # gets loads needed later.
            load_blocked(nc.sync, wq_all, wq_d, C, KT)
            load_blocked(nc.scalar, wk_all, wk_d, C, KT)
            load_blocked(nc.gpsimd, xt_all, xt_d, N, KT, 0 * 512, 512)
            nc.sync.dma_start(cos_sb[:], cos_d[:])
            nc.scalar.dma_start(sin_sb[:], sin_d[:])
            load_blocked(nc.gpsimd, wv_all, wv_d, C, KT)
            load_blocked(nc.sync, xt_all, xt_d, N, KT, 1 * 512, 512)
            nc.scalar.dma_start(tri_sb[:], tri_d[:])
            nc.scalar.dma_start(id_sb[:], id_d[:])
            nc.scalar.dma_start(sel_sb[:], sel_d[:])
            load_blocked(nc.gpsimd, xt_all, xt_d, N, KT, 2 * 512, 512)
            load_blocked(nc.sync, xt_all, xt_d, N, KT, 3 * 512, 512)
            load_blocked(nc.gpsimd, wout_all, wout_d, D, 2)

            def xtk(k, a, b):
                return xt_all[:, k * N + a:k * N + b]

            # ---------------- persistent SBUF results
            qr_sb = [pp.tile([128, N], BF16, tag=f"qr{t}", name=f"qr{t}")
                     for t in range(2)]
            kr_sb = [pp.tile([128, N], BF16, tag=f"kr{t}", name=f"kr{t}")
                     for t in range(2)]
            vaug_sb = [pp.tile([128, HPG * (HD + 1)], BF16, tag=f"va{i}",
                               name=f"va{i}") for i in range(NT)]
            ou_sb = [pp.tile([128, N], BF16, tag=f"ou{t}", name=f"ou{t}")
                     for t in range(2)]
            o_sb = [pp.tile([128, N], BF16, tag=f"o{t}", name=f"o{t}")
                    for t in range(2)]
            sums_sb = [pp.tile([4, 512], F32, tag=f"sums{qc}", name=f"sums{qc}")
                       for qc in range(NCH)]
            # ones columns of vaug written once
            for i in range(NT):
                ap = vaug_sb[i][:]
                dst1 = AP(ap.tensor, ap.offset + HD,
                          [[HPG * (HD + 1), 128], [HD + 1, HPG]])
                nc.gpsimd.memset(dst1, 1.0)

            # ---------------- PE work generators
            def gen_qk(w_all, dst, mt, ch):
                ps = fpool.tile([128, 512], F32, tag="fill", name="qkps")
                for k in range(KT):
                    nc.tensor.matmul(
                        ps[:],
                        w_all[:, k * C + mt * 128:k * C + (mt + 1) * 128],
                        xtk(k, ch * 512, (ch + 1) * 512),
                        start=(k == 0), stop=(k == KT - 1))
                    yield
                cs = cos_sb[:, ch * 512:(ch + 1) * 512]
                sn = sin_sb[:, ch * 512:(ch + 1) * 512]
                xs = sp.tile([128, 512], F32, tag="xs", name="xs", bufs=2)
                nc.vector.stream_shuffle(xs[:], ps[:], _SHUF_MASK)
                m2 = sp.tile([128, 512], F32, tag="mm", name="m2")
                nc.vector.tensor_mul(m2[:], xs[:], sn)
                m1 = sp.tile([128, 512], F32, tag="mm", name="m1")
                nc.vector.tensor_mul(m1[:], ps[:], cs)
                nc.vector.tensor_add(
                    dst[mt][:, ch * 512:(ch + 1) * 512], m1[:], m2[:])

            def gen_v(i):
                ps = fpool.tile([128, C], F32, tag="fill", name="vps")
                for k in range(KT):
                    nc.tensor.matmul(
                        ps[:], xtk(k, i * 128, (i + 1) * 128),
                        wv_all[:, k * C:(k + 1) * C],
                        start=(k == 0), stop=(k == KT - 1))
                    yield
                ap = vaug_sb[i][:]
                dst = AP(ap.tensor, ap.offset,
                         [[HPG * (HD + 1), 128], [HD + 1, HPG], [1, HD]])
                # gpsimd cannot read PSUM: scalar is idle during phase A
                # (tiles 0-7), DVE takes the mid-attention ones (8-15)
                if i < 8:
                    nc.scalar.copy(dst, ps[:].rearrange("p (a c) -> p a c",
                                                        a=HPG, c=HD))
                else:
                    nc.vector.tensor_copy(dst, ps[:].rearrange(
                        "p (a c) -> p a c", a=HPG, c=HD))

            def gen_norm(qc):
                # recip + cast emitted immediately (DVE/scalar); the PE bc
                # matmuls + DVE muls come as two filler quanta.
                rrf = sp.tile([4, 512], F32, tag="rrf", name="rrf", bufs=2)
                nc.vector.reciprocal_approx_fast(rrf[:], sums_sb[qc][:])
                rr = sp.tile([4, 512], F32R, tag="rr", name="rr", bufs=2)
                nc.scalar.copy(rr[:], rrf[:])
                for t in range(2):
                    yield
                    bc = fpool.tile([128, 512], F32, tag="fill", name="bc")
                    nc.tensor.matmul(bc[:], sel_sb[:, t * 128:(t + 1) * 128],
                                     rr[:], start=True, stop=True)
                    nc.vector.tensor_mul(
                        o_sb[t][:, qc * 512:(qc + 1) * 512],
                        ou_sb[t][:, qc * 512:(qc + 1) * 512], bc[:])

            def gen_op(qc, ilist):
                for i in ilist:
                    for cc in range(2):
                        ps = fpool.tile([128, 512], F32, tag="fill", name="ops")
                        for t in range(2):
                            nc.tensor.matmul(
                                ps[:],
                                o_sb[t][:, i * 128:(i + 1) * 128],
                                wout_all[:, t * D + cc * 512:
                                         t * D + (cc + 1) * 512],
                                start=(t == 0), stop=(t == 1))
                        oc = sp.tile([128, 512], F32, tag="oc", name="oc",
                                     bufs=3)
                        # qc 3 runs in the scalar-idle tail: alternate
                        # scalar/vector there so the copies overlap
                        if qc == 3 and (i * 2 + cc) % 2 == 0:
                            nc.scalar.copy(oc[:], ps[:])
                        else:
                            nc.vector.tensor_copy(oc[:], ps[:])
                        # alternate output-DMA queues so the final drain
                        # isn't serialized on one queue
                        dma_h = nc.sync if (i * 2 + cc) % 2 == 0 else nc.gpsimd
                        dma_h.dma_start(
                            out_d[i * 128:(i + 1) * 128,
                                  cc * 512:(cc + 1) * 512], oc[:])
                        yield

            # filler machinery: list of generators, pulled n quanta at a time
            def mk_fill(gens):
                gens = list(gens)
                def fill(n):
                    while n > 0 and gens:
                        try:
                            next(gens[0])
                            n -= 1
                        except StopIteration:
                            gens.pop(0)
                def drain():
                    while gens:
                        try:
                            next(gens[0])
                        except StopIteration:
                            gens.pop(0)
                return fill, drain

            def run(gen):  # run a generator to completion (dense emission)
                for _ in gen:
                    pass

            # ---------------- attention head with PV lag + fillers
            def attention_head(qc, hl, fill, rate=2):
                t = hl // 2
                pb = (hl % 2) * 64
                nmt = 4 * (qc + 1)
                pv = pvp.tile([HD + 1, 512], F32, tag="pv", name="pv")
                pend = []

                def emit_pv(e_sb, q0, mt):
                    nc.tensor.matmul(
                        pv[:, q0:512],
                        vaug_sb[mt][:, hl * (HD + 1):(hl + 1) * (HD + 1)],
                        e_sb[:, q0:512],
                        start=(mt == 0), stop=(mt == nmt - 1))

                for mt in range(nmt):
                    v = mt - 4 * qc          # >=0 on diagonal m-tiles
                    q0 = 128 * v if v > 0 else 0
                    s_ps = spool.tile([128, 512], F32, tag="s", name="sps")
                    nc.tensor.matmul(
                        s_ps[:, q0:512],
                        kr_sb[t][pb:pb + 64, mt * 128:(mt + 1) * 128],
                        qr_sb[t][pb:pb + 64, qc * 512 + q0:(qc + 1) * 512],
                        start=True, stop=(v < 0))
                    if v >= 0:
                        nc.tensor.matmul(
                            s_ps[:, q0:q0 + 128], id_sb[:], tri_sb[:],
                            start=False, stop=True)
                    e_sb = sp.tile([128, 512], BF16, tag="e", name="e", bufs=6)
                    nc.scalar.activation(e_sb[:, q0:512], s_ps[:, q0:512],
                                         EXP, scale=SCALE)
                    pend.append((e_sb, q0, mt))
                    if len(pend) > LAG:
                        emit_pv(*pend.pop(0))
                        fill(rate)
                while pend:
                    emit_pv(*pend.pop(0))
                    fill(rate)
                # evacuate sums row + unnormalized output (DVE; gpsimd has no
                # PSUM access, engines can only write partition bases
                # 0/32/64/96 so the row bounces through partition 0 + DMA)
                sr = sp.tile([1, 512], F32, tag="sr", name="sr", bufs=2)
                nc.vector.tensor_copy(sr[:], pv[64:65, :])
                nc.sync.dma_start(sums_sb[qc][hl:hl + 1, :], sr[:])
                nc.vector.tensor_copy(
                    ou_sb[t][pb:pb + 64, qc * 512:(qc + 1) * 512],
                    pv[0:64, :])

            def gen_delay(n):
                for _ in range(n):
                    yield

            # ---------------- phase A: dense warm-up touching only xt ch0
            # first, then ch1, so At[1, heads 0/1] can start ASAP
            run(gen_qk(wq_all, qr_sb, 0, 0))
            run(gen_qk(wk_all, kr_sb, 0, 0))
            for i in range(4):
                run(gen_v(i))
            run(gen_qk(wq_all, qr_sb, 0, 1))
            run(gen_qk(wk_all, kr_sb, 0, 1))
            for i in range(4, 8):
                run(gen_v(i))

            # ---------------- interleaved main phases; q-chunk order 1,2,3
            # with the small qc=0 heads woven into the under-filled qc=3
            # phase, so the only tail is norm3+op3
            plan = {
                (1, 0): [gen_qk(wq_all, qr_sb, 1, 0), gen_qk(wk_all, kr_sb, 1, 0)],
                (1, 1): [gen_qk(wq_all, qr_sb, 1, 1), gen_qk(wk_all, kr_sb, 1, 1)],
                (1, 2): [gen_qk(wq_all, qr_sb, 0, 2), gen_qk(wk_all, kr_sb, 0, 2)],
                (1, 3): [gen_qk(wq_all, qr_sb, 1, 2), gen_qk(wk_all, kr_sb, 1, 2)],
                (2, 0): [gen_v(8), gen_v(9), gen_v(10), gen_v(11), "norm1"],
                (2, 1): [gen_qk(wq_all, qr_sb, 0, 3), gen_qk(wk_all, kr_sb, 0, 3)],
                (2, 2): [gen_qk(wq_all, qr_sb, 1, 3), gen_qk(wk_all, kr_sb, 1, 3),
                         "op1a"],
                (2, 3): ["op1b"],
                (3, 0): [gen_v(12), gen_v(13), gen_v(14), gen_v(15), "norm2"],
                (0, 0): ["op2a"],
                (3, 1): ["op2b"],
                (0, 1): [],
                (0, 2): [],
                (0, 3): [],
                (3, 2): [gen_delay(6), "norm0"],
                (3, 3): ["op0a", "op0b"],
            }
            named = {
                "norm0": lambda: gen_norm(0),
                "norm1": lambda: gen_norm(1),
                "norm2": lambda: gen_norm(2),
                "op0a": lambda: gen_op(0, [0, 1]),
                "op0b": lambda: gen_op(0, [2, 3]),
                "op1a": lambda: gen_op(1, [4, 5]),
                "op1b": lambda: gen_op(1, [6, 7]),
                "op2a": lambda: gen_op(2, [8, 9]),
                "op2b": lambda: gen_op(2, [10, 11]),
            }
            # higher pull-rate where same-head V-proj fillers must finish
            # before the pv matmuls that consume them (see deadlock audit)
            rates = {(2, 0): 4, (3, 0): 3, (3, 2): 1}
            order = [(1, 0), (1, 1), (1, 2), (1, 3),
                     (2, 0), (2, 1), (2, 2), (2, 3),
                     (3, 0), (0, 0), (3, 1), (0, 1),
                     (0, 2), (0, 3), (3, 2), (3, 3)]
            for qc, hl in order:
                gens = [named[g]() if isinstance(g, str) else g
                        for g in plan[(qc, hl)]]
                fill, drain = mk_fill(gens)
                attention_head(qc, hl, fill, rate=rates.get((qc, hl), 2))
                drain()
            # tail: normalize + project chunk 3
            run(gen_norm(3))
            run(gen_op(3, [12, 13, 14, 15]))

    nc.compile()
    return nc


# ---------------------------------------------------------------- host wrapper

_NC = None


def make_in_maps(X, Wqkv, Wout, bout):
    X = np.ascontiguousarray(np.asarray(X, np.float32))
    Wqkv = np.asarray(Wqkv, np.float32)
    Wout = np.asarray(Wout, np.float32)
    in_maps = []
    for core in range(8):
        b, g = core // 4, core % 4
        heads = [HPG * g + hl for hl in range(HPG)]
        qcols = np.concatenate([h * HD + _PERM for h in heads])
        vcols = np.concatenate([h * HD + np.arange(HD) for h in heads])
        in_maps.append({
            "xt": np.ascontiguousarray(X[b].T).astype(ml_dtypes.bfloat16),
            "wq": np.ascontiguousarray(Wqkv[:, qcols]).astype(ml_dtypes.bfloat16),
            "wk": np.ascontiguousarray(Wqkv[:, 1024 + qcols]).astype(ml_dtypes.bfloat16),
            "wv": np.ascontiguousarray(Wqkv[:, 2048 + vcols]).astype(ml_dtypes.bfloat16),
            "wout": np.ascontiguousarray(Wout[vcols, :]).astype(ml_dtypes.bfloat16),
            "cos2": _COS2.astype(ml_dtypes.bfloat16),
            "sin2": _SIN2.astype(ml_dtypes.bfloat16),
            "tri": _TRI.astype(ml_dtypes.bfloat16),
            "ident": _IDENT.astype(ml_dtypes.bfloat16),
            "sel": _SEL,
        })
    return in_maps


def assemble(results, bout):
    out = np.zeros((B, N, D), np.float32)
    for core in range(8):
        out[core // 4] += results[core]["out"]
    out += np.asarray(bout, np.float32)[None, None, :]
    return out


def kernel(X, Wqkv, Wout, bout):
    global _NC
    from concourse import bass_utils
    if _NC is None:
        _NC = build_nc()
    in_maps = make_in_maps(X, Wqkv, Wout, bout)
    res = bass_utils.run_bass_kernel_spmd(_NC, in_maps, core_ids=list(range(8)))
    return assemble(res.results, bout)
